# revision 1
# baseline (speedup 1.0000x reference)
import numpy as np
from contextlib import ExitStack

import concourse.bacc as bacc
import concourse.tile as tile
from concourse import mybir
from concourse.bass_utils import run_bass_kernel_spmd

B, T, D = 16, 4096, 1024
N_CORES = 8
T_SHARD = T // N_CORES          # 512 timesteps per core
P = 128                         # SBUF partitions
Q = T_SHARD // P                # 4 t-rows packed per partition
SCALE = 32.0                    # sqrt(D) = sqrt(1024)
F32 = mybir.dt.float32

_compiled = None


def _make_pe() -> np.ndarray:
    pos = np.arange(T, dtype=np.float32)[:, None]
    i_even = np.arange(0, D, 2, dtype=np.float32)
    div_sin = np.power(np.float32(10000.0), np.float32(2.0) * i_even / np.float32(D))
    div_cos = np.power(
        np.float32(10000.0), np.float32(2.0) * (i_even + np.float32(1.0)) / np.float32(D)
    )
    pe = np.zeros((T, D), dtype=np.float32)
    pe[:, 0::2] = np.sin(pos / div_sin)
    pe[:, 1::2] = np.cos(pos / div_cos)
    return pe


def _build():
    global _compiled
    if _compiled is not None:
        return _compiled

    nc = bacc.Bacc("TRN2", target_bir_lowering=False, debug=False, num_devices=N_CORES)
    x_dram = nc.dram_tensor("x", [B, T_SHARD, D], F32, kind="ExternalInput").ap()
    pe_dram = nc.dram_tensor("pe", [T_SHARD, D], F32, kind="ExternalInput").ap()
    out_dram = nc.dram_tensor("out", [B, T_SHARD, D], F32, kind="ExternalOutput").ap()

    with tile.TileContext(nc) as tc, ExitStack() as ctx:
        pe_pool = ctx.enter_context(tc.tile_pool(name="pe", bufs=1))
        x_pool = ctx.enter_context(tc.tile_pool(name="x", bufs=4))
        o_pool = ctx.enter_context(tc.tile_pool(name="o", bufs=4))

        pe_tile = pe_pool.tile([P, Q * D], F32)
        nc.sync.dma_start(pe_tile[:], pe_dram.rearrange("(p q) d -> p (q d)", p=P))

        for b in range(B):
            xt = x_pool.tile([P, Q * D], F32)
            nc.sync.dma_start(xt[:], x_dram[b].rearrange("(p q) d -> p (q d)", p=P))
            ot = o_pool.tile([P, Q * D], F32)
            nc.vector.scalar_tensor_tensor(
                out=ot[:],
                in0=xt[:],
                scalar=SCALE,
                in1=pe_tile[:],
                op0=mybir.AluOpType.mult,
                op1=mybir.AluOpType.add,
            )
            nc.scalar.dma_start(
                out_dram[b].rearrange("(p q) d -> p (q d)", p=P), ot[:]
            )

    nc.compile()
    _compiled = nc
    return nc


def kernel(x: np.ndarray, **run_kwargs) -> np.ndarray:
    nc = _build()
    pe = _make_pe()
    in_maps = []
    for c in range(N_CORES):
        t0 = c * T_SHARD
        in_maps.append(
            {
                "x": np.ascontiguousarray(x[:, t0 : t0 + T_SHARD, :], dtype=np.float32),
                "pe": np.ascontiguousarray(pe[t0 : t0 + T_SHARD, :]),
            }
        )
    res = run_bass_kernel_spmd(nc, in_maps, core_ids=list(range(N_CORES)), **run_kwargs)
    out = np.concatenate([res.results[c]["out"] for c in range(N_CORES)], axis=1)
    if run_kwargs.get("trace"):
        kernel.last_exec_time_ns = res.exec_time_ns
        kernel.last_results = res
    return out


# revision 3
# speedup vs baseline: 1.0388x; 1.0388x over previous
import numpy as np
from contextlib import ExitStack

import concourse.bacc as bacc
import concourse.tile as tile
from concourse import mybir
from concourse.bass_utils import run_bass_kernel_spmd

B, T, D = 16, 4096, 1024
N_CORES = 8
T_SHARD = T // N_CORES          # 512 timesteps per core
P = 128                         # SBUF partitions
Q = T_SHARD // P                # 4 t-rows packed per partition
SCALE = 32.0                    # sqrt(D) = sqrt(1024)
F32 = mybir.dt.float32

_compiled = None


def _make_pe() -> np.ndarray:
    pos = np.arange(T, dtype=np.float32)[:, None]
    i_even = np.arange(0, D, 2, dtype=np.float32)
    div_sin = np.power(np.float32(10000.0), np.float32(2.0) * i_even / np.float32(D))
    div_cos = np.power(
        np.float32(10000.0), np.float32(2.0) * (i_even + np.float32(1.0)) / np.float32(D)
    )
    pe = np.zeros((T, D), dtype=np.float32)
    pe[:, 0::2] = np.sin(pos / div_sin)
    pe[:, 1::2] = np.cos(pos / div_cos)
    return pe


def _build():
    global _compiled
    if _compiled is not None:
        return _compiled

    nc = bacc.Bacc("TRN2", target_bir_lowering=False, debug=False, num_devices=N_CORES)
    x_dram = nc.dram_tensor("x", [B, T_SHARD, D], F32, kind="ExternalInput").ap()
    pe_dram = nc.dram_tensor("pe", [T_SHARD, D], F32, kind="ExternalInput").ap()
    out_dram = nc.dram_tensor("out", [B, T_SHARD, D], F32, kind="ExternalOutput").ap()

    with tile.TileContext(nc) as tc, ExitStack() as ctx:
        pe_pool = ctx.enter_context(tc.tile_pool(name="pe", bufs=1))
        x_pool = ctx.enter_context(tc.tile_pool(name="x", bufs=3))
        o_pool = ctx.enter_context(tc.tile_pool(name="o", bufs=2))

        pe_tile = pe_pool.tile([P, Q * D], F32)
        nc.sync.dma_start(pe_tile[:], pe_dram.rearrange("(p q) d -> p (q d)", p=P))

        QD = Q * D
        for j in range(B // 2):
            xt = x_pool.tile([P, 2 * QD], F32)
            nc.sync.dma_start(
                xt[:].rearrange("p (b f) -> p b f", b=2),
                x_dram[2 * j : 2 * j + 2].rearrange("b (p q) d -> p b (q d)", p=P),
            )
            ot = o_pool.tile([P, 2 * QD], F32)
            for h in range(2):
                nc.vector.scalar_tensor_tensor(
                    out=ot[:, h * QD : (h + 1) * QD],
                    in0=xt[:, h * QD : (h + 1) * QD],
                    scalar=SCALE,
                    in1=pe_tile[:],
                    op0=mybir.AluOpType.mult,
                    op1=mybir.AluOpType.add,
                )
            nc.scalar.dma_start(
                out_dram[2 * j : 2 * j + 2].rearrange("b (p q) d -> p b (q d)", p=P),
                ot[:].rearrange("p (b f) -> p b f", b=2),
            )

    nc.compile()
    _compiled = nc
    return nc


def kernel(x: np.ndarray, **run_kwargs) -> np.ndarray:
    nc = _build()
    pe = _make_pe()
    in_maps = []
    for c in range(N_CORES):
        t0 = c * T_SHARD
        in_maps.append(
            {
                "x": np.ascontiguousarray(x[:, t0 : t0 + T_SHARD, :], dtype=np.float32),
                "pe": np.ascontiguousarray(pe[t0 : t0 + T_SHARD, :]),
            }
        )
    res = run_bass_kernel_spmd(nc, in_maps, core_ids=list(range(N_CORES)), **run_kwargs)
    out = np.concatenate([res.results[c]["out"] for c in range(N_CORES)], axis=1)
    if run_kwargs.get("trace"):
        kernel.last_exec_time_ns = res.exec_time_ns
        kernel.last_results = res
    return out


# revision 5
# speedup vs baseline: 1.1735x; 1.1296x over previous
import numpy as np
from contextlib import ExitStack

import concourse.bacc as bacc
import concourse.tile as tile
from concourse import mybir
from concourse.bass_utils import run_bass_kernel_spmd

B, T, D = 16, 4096, 1024
N_CORES = 8
T_SHARD = T // N_CORES          # 512 timesteps per core
P = 128                         # SBUF partitions
Q = T_SHARD // P                # 4 t-rows packed per partition
SCALE = 32.0                    # sqrt(D) = sqrt(1024)
F32 = mybir.dt.float32

_compiled = None


def _make_pe() -> np.ndarray:
    pos = np.arange(T, dtype=np.float32)[:, None]
    i_even = np.arange(0, D, 2, dtype=np.float32)
    div_sin = np.power(np.float32(10000.0), np.float32(2.0) * i_even / np.float32(D))
    div_cos = np.power(
        np.float32(10000.0), np.float32(2.0) * (i_even + np.float32(1.0)) / np.float32(D)
    )
    pe = np.zeros((T, D), dtype=np.float32)
    pe[:, 0::2] = np.sin(pos / div_sin)
    pe[:, 1::2] = np.cos(pos / div_cos)
    return pe


def _build():
    global _compiled
    if _compiled is not None:
        return _compiled

    nc = bacc.Bacc("TRN2", target_bir_lowering=False, debug=False, num_devices=N_CORES)
    x_dram = nc.dram_tensor("x", [B, T_SHARD, D], F32, kind="ExternalInput").ap()
    pe_dram = nc.dram_tensor("pe", [T_SHARD, D], F32, kind="ExternalInput").ap()
    out_dram = nc.dram_tensor("out", [B, T_SHARD, D], F32, kind="ExternalOutput").ap()

    with tile.TileContext(nc) as tc, ExitStack() as ctx:
        pe_pool = ctx.enter_context(tc.tile_pool(name="pe", bufs=1))
        x_pool = ctx.enter_context(tc.tile_pool(name="x", bufs=5))

        pe_tile = pe_pool.tile([P, Q * D], F32)
        nc.sync.dma_start(pe_tile[:], pe_dram.rearrange("(p q) d -> p (q d)", p=P))

        QD = Q * D
        for j in range(B // 2):
            xt = x_pool.tile([P, 2 * QD], F32)
            nc.sync.dma_start(
                xt[:].rearrange("p (b f) -> p b f", b=2),
                x_dram[2 * j : 2 * j + 2].rearrange("b (p q) d -> p b (q d)", p=P),
            )
            for h in range(2):
                nc.vector.scalar_tensor_tensor(
                    out=xt[:, h * QD : (h + 1) * QD],
                    in0=xt[:, h * QD : (h + 1) * QD],
                    scalar=SCALE,
                    in1=pe_tile[:],
                    op0=mybir.AluOpType.mult,
                    op1=mybir.AluOpType.add,
                )
            nc.scalar.dma_start(
                out_dram[2 * j : 2 * j + 2].rearrange("b (p q) d -> p b (q d)", p=P),
                xt[:].rearrange("p (b f) -> p b f", b=2),
            )

    nc.compile()
    _compiled = nc
    return nc


def kernel(x: np.ndarray, **run_kwargs) -> np.ndarray:
    nc = _build()
    pe = _make_pe()
    in_maps = []
    for c in range(N_CORES):
        t0 = c * T_SHARD
        in_maps.append(
            {
                "x": np.ascontiguousarray(x[:, t0 : t0 + T_SHARD, :], dtype=np.float32),
                "pe": np.ascontiguousarray(pe[t0 : t0 + T_SHARD, :]),
            }
        )
    res = run_bass_kernel_spmd(nc, in_maps, core_ids=list(range(N_CORES)), **run_kwargs)
    out = np.concatenate([res.results[c]["out"] for c in range(N_CORES)], axis=1)
    if run_kwargs.get("trace"):
        kernel.last_exec_time_ns = res.exec_time_ns
        kernel.last_results = res
    return out
